# revision 50
# baseline (speedup 1.0000x reference)
"""Trainium2 Bass kernel for virtual-node GAT attention (gnn_message_passing).

Reference semantics (N=100000, C=64, D=512, F=256):
    gh  = graph_node @ W            # (N, F)
    vh  = virtual_node @ W          # (C, F)
    e   = gh @ a1 + (vh @ a2)^T     # (N, C)
    e   = leaky_relu(e, 0.2)
    att = softmax(e, axis=1)
    out = att @ vh                  # (N, F)

Algebraic identity: gh only enters via gh @ a1 = graph_node @ (W @ a1), so
the (N,D)@(D,F) matmul never happens. Host precomputes the tiny shared
tables w1 = W@a1 (D,), vh (C,F), t = vh@a2 (C,). The kernel is HBM-bound:
streaming x in and h' out once. Both streams ride bf16 (host casts), which
halves HBM traffic vs fp32; rel-err budget (2e-2) dwarfs bf16 noise.

Device pipeline, per 512-row block (x shipped TRANSPOSED by the host as
[4 d-chunks, 128, rows] bf16):
  PE   e^T[j, r] = sum_d w1[d] x[r, d]: 4 accumulating matmuls with
       lhsT = (w1 chunk) replicated across 64 columns, rhs = xT chunk.
       Output lands already transposed for the att matmul (no PE transpose,
       no DVE dot product). Two blocks pack one PSUM bank (partitions
       0-63 / 64-127).
  ACT  e = prelu(e^T + t) (bias=t fused, alpha honored), then exp -> bf16.
  PE   h'[r, :] = att^T.T @ [vh | 1]: the ones column makes col 256 the
       softmax denominator z (no reduction pass).
  DVE  r = 1/z; normalization fused into the PSUM->SBUF copies (split
       between ACT and DVE), writing bf16.

Host column permutation: xT column rc*128+rp holds row 4*rp+rc of its
block, so each h'-matmul output partition owns 4 consecutive HBM rows ->
2KB contiguous store packets and natural row order (no un-permute).

Sharding: rows split evenly across 8 cores (data parallel); small tables
replicated; no cross-device communication.
"""

import numpy as np

N, D, F, C = 100000, 512, 256, 64
NCORES = 8
SHARD = N // NCORES            # 12500 rows per core
P = 128
BLK = 512                      # rows per block (4 psum chunks of 128)
NBLK = 25                      # ceil(12500 / 512)
PADROWS = NBLK * BLK           # 12800
# Pairs of blocks share one PSUM logits bank: (0,1), (2,3), ..., (24,).
# Full pairs stripe 8 rows/partition -> 4KB store packets; the trailing
# singleton block keeps 4-row striping (2KB packets).
GROUPS_IN = [1, 2, 4, 6, 6, 6]   # blocks per input DMA instruction
GROUPS_OUT_PAIRS = [1, 2, 3, 3, 2, 1]  # PAIRS per output DMA + tail block
assert sum(GROUPS_IN) == NBLK and sum(GROUPS_OUT_PAIRS) == NBLK // 2
CONST_BYTES = 2 * 4 * C + 4 + 2 * (F + 1) + 2   # 1032 (pad: pitch % 4 == 0)
ALPHA = 0.2

_CACHE = {}


def _build_nc():
    import concourse.bacc as bacc
    import concourse.mybir as mybir
    import concourse.tile as tile

    fp32 = mybir.dt.float32
    bf16 = mybir.dt.bfloat16
    fp16 = mybir.dt.float16
    Act = mybir.ActivationFunctionType
    Alu = mybir.AluOpType

    nc = bacc.Bacc("TRN2", target_bir_lowering=False, debug=False,
                   num_devices=NCORES)
    # x and w1 ride fp16 (same bytes as bf16, 8x finer mantissa -> the
    # logits see ~8x less quantization noise). pexp/vha stay bf16: exp can
    # reach ~5e8 which overflows fp16.
    xT = nc.dram_tensor("xT", [4, P, PADROWS], fp16, kind="ExternalInput").ap()
    # wrep fp16 [P,4,C] | tcol fp32 [P,1] | vha bf16 [P,F+1], packed as bytes
    consts = nc.dram_tensor("consts", [P, CONST_BYTES], mybir.dt.uint8,
                            kind="ExternalInput").ap()
    out = nc.dram_tensor("out", [PADROWS, F], bf16, kind="ExternalOutput").ap()

    # block -> (input group idx, local block idx); pair -> output group
    gin_of, gout_of = {}, {}
    b = 0
    for g, gs in enumerate(GROUPS_IN):
        for i in range(gs):
            gin_of[b] = (g, i)
            b += 1
    pr = 0
    for g, gs in enumerate(GROUPS_OUT_PAIRS):
        for i in range(gs):
            gout_of[pr] = (g, i)
            pr += 1
    gin_row0 = np.cumsum([0] + GROUPS_IN)
    gout_row0 = np.cumsum([0] + GROUPS_OUT_PAIRS)

    with tile.TileContext(nc) as tc:
        with (
            tc.tile_pool(name="const", bufs=1) as constp,
            tc.tile_pool(name="xin", bufs=3) as xp,
            tc.tile_pool(name="esb", bufs=3) as ep,
            tc.tile_pool(name="pexp", bufs=4) as pexpp,
            tc.tile_pool(name="rvec", bufs=6) as rp_,
            tc.tile_pool(name="osb", bufs=3) as op_,
            tc.tile_pool(name="psE", bufs=2, space="PSUM") as psE,
            tc.tile_pool(name="psH", bufs=3, space="PSUM") as psH,
        ):
            # all consts ride ONE dma on the ACT HWDGE ring (the x stream
            # owns the SP ring); typed views are bitcasts of the byte tile
            cst = constp.tile([P, CONST_BYTES], mybir.dt.uint8, name="cst")
            nc.scalar.dma_start(out=cst, in_=consts)
            wrep_sb = cst[:, 0:512].bitcast(fp16).rearrange(
                "p (c j) -> p c j", c=4)
            tcol_sb = cst[:, 512:516].bitcast(fp32)
            vha_sb = cst[:, 516:516 + 2 * (F + 1)].bitcast(bf16)

            # zeroed tile for PE-warming filler matmuls (see front())
            warm16 = constp.tile([P, BLK], fp16, name="warm16")
            nc.vector.memset(warm16, 0.0)



            xt_tiles = [None] * len(GROUPS_IN)
            osb_tiles = [None] * len(GROUPS_OUT_PAIRS)

            def ensure_xt(b):
                g, _ = gin_of[b]
                if xt_tiles[g] is None:
                    gs = GROUPS_IN[g]
                    t = xp.tile([P, 4, gs * BLK], fp16, tag="xt", name="xt")
                    src = xT[:, :, gin_row0[g] * BLK:(gin_row0[g] + gs) * BLK]
                    nc.sync.dma_start(out=t, in_=src.rearrange("c p r -> p c r"))
                    xt_tiles[g] = t
                return xt_tiles[g], gin_of[b][1]

            def ensure_osb(pi):
                g, _ = gout_of[pi]
                if osb_tiles[g] is None:
                    gs = GROUPS_OUT_PAIRS[g]
                    osb_tiles[g] = op_.tile([P, gs, 8, F], bf16, tag="osb",
                                            name="osb")
                return osb_tiles[g], gout_of[pi][1]

            pairs = [(b, b + 1) for b in range(0, NBLK - 1, 2)] + [(NBLK - 1,)]
            pex_of = {}

            def front(pi):
                # e^T matmuls + prelu + exp for pair pi. Emitted one pair
                # AHEAD of the back half so the next prelu/exp sit in the
                # ACT FIFO before this pair's copies — keeps the PE fed and
                # the HAM clock-gate warm.
                pair = pairs[pi]
                nh = len(pair)               # blocks in this psum pair
                npart = nh * C               # active psum partitions
                pse = psE.tile([P, BLK], fp32, name="pse", tag="pse")
                # PE-warming fillers: absorb the PE's wait-for-DMA/ACT gap
                # so the HAM clock gate never sees an idle window and the
                # array stays at 2.4 GHz. They write garbage into pse that
                # the first real matmul's start=True immediately resets.
                for _ in range(3):
                    nc.tensor.matmul(pse[:C, :F], warm16[:, :C],
                                     warm16[:, :F], start=True, stop=True)
                for h in range(nh):
                    xt, lb = ensure_xt(pair[h])
                    for dc in range(4):
                        nc.tensor.matmul(
                            pse[h * C:(h + 1) * C, :],
                            wrep_sb[:, dc, :],
                            xt[:, dc, lb * BLK:(lb + 1) * BLK],
                            start=(dc == 0), stop=(dc == 3))
                esb = ep.tile([P, BLK], fp32, tag="esb", name="esb")
                nc.scalar.activation(
                    out=esb[:npart, :], in_=pse[:npart, :], func=Act.Prelu,
                    bias=tcol_sb[:npart, :], scale=1.0, alpha=ALPHA)
                pex = pexpp.tile([P, BLK], bf16, tag="pex", name="pex")
                nc.scalar.activation(out=pex[:npart, :], in_=esb[:npart, :],
                                     func=Act.Exp)
                pex_of[pi] = pex

            def back(pi):
                pair = pairs[pi]
                nh = len(pair)
                pex = pex_of.pop(pi)
                if nh == 2:
                    osb, ob = ensure_osb(pi)
                else:
                    # trailing singleton block: own 4-row-striped mini tile
                    osb = op_.tile([P, 1, 4, F], bf16, tag="osbt",
                                   name="osbt")
                    ob = 0
                for h in range(nh):
                    b = pair[h]
                    for cc in range(2):
                        ph = psH.tile([P, 2, BLK], fp32, name="ph", tag="ph")
                        for i in range(2):
                            rc = cc * 2 + i
                            nc.tensor.matmul(
                                ph[:, i, :F + 1],
                                pex[h * C:(h + 1) * C, rc * P:(rc + 1) * P],
                                vha_sb[h * C:(h + 1) * C, :],
                                start=True, stop=True)
                        r2 = rp_.tile([P, 2, 1], fp32)
                        nc.vector.reciprocal(r2[:, :, 0], ph[:, :, F])
                        # normalize during PSUM->SBUF copy. One tile per
                        # pair drains via ACT (2 per-chunk scaled copies);
                        # the rest drain via a single DVE pass each, with
                        # 1/z broadcast along the free dim
                        k = 2 * h + cc
                        slot = 4 * h + cc * 2
                        if k == 0:
                            nc.scalar.mul(osb[:, ob, slot, :],
                                          ph[:, 0, :F], r2[:, 0, :])
                            nc.scalar.mul(osb[:, ob, slot + 1, :],
                                          ph[:, 1, :F], r2[:, 1, :])
                        else:
                            nc.vector.scalar_tensor_tensor(
                                out=osb[:, ob, slot:slot + 2, :],
                                in0=ph[:, :, :F], scalar=1.0,
                                in1=r2.broadcast_to([P, 2, F]),
                                op0=Alu.mult, op1=Alu.mult)
                if nh == 2:
                    g, ob2 = gout_of[pi]
                    if ob2 == GROUPS_OUT_PAIRS[g] - 1:
                        gs = GROUPS_OUT_PAIRS[g]
                        r0 = gout_row0[g] * 2 * BLK
                        dst = out[r0:r0 + gs * 2 * BLK, :]
                        nc.scalar.dma_start(
                            out=dst.rearrange("(pr p eight) f -> p pr eight f",
                                              eight=8, p=P),
                            in_=osb_tiles[g])
                else:
                    dst = out[(NBLK - 1) * BLK:NBLK * BLK, :]
                    nc.scalar.dma_start(
                        out=dst.rearrange("(b p four) f -> p b four f",
                                          four=4, p=P),
                        in_=osb)

            # depth-1 software pipeline: exp(k+1) sits ahead of pair k's
            # copies in the ACT FIFO so the PE's h' matmuls don't wait
            npairs = len(pairs)
            front(0)
            for pi in range(1, npairs):
                front(pi)
                back(pi - 1)
            back(npairs - 1)

    nc.compile()
    return nc


def _get_nc():
    if "nc" not in _CACHE:
        _CACHE["nc"] = _build_nc()
    return _CACHE["nc"]


def _prep_inputs(graph_node, virtual_node, W, a):
    import ml_dtypes
    f32 = np.float32
    bf16 = ml_dtypes.bfloat16
    W = np.asarray(W, f32)
    a = np.asarray(a, f32)
    a1 = a[:F, 0]
    a2 = a[F:, 0]
    w1 = (W @ a1).astype(f32)                             # (D,)
    vh = (np.asarray(virtual_node, f32) @ W).astype(f32)  # (C, F)
    t = (vh @ a2).astype(f32)                             # (C,)

    # wrep[p, dc, j] = w1[dc*128 + p] for all j (broadcast across columns)
    wrep = np.ascontiguousarray(
        np.broadcast_to(w1.reshape(4, P).T[:, :, None], (P, 4, C))
    ).astype(np.float16)
    tcol = np.ascontiguousarray(np.concatenate([t, t])[:, None], dtype=f32)
    vha = np.ones((P, F + 1), f32)
    vha[:C, :F] = vh
    vha[C:, :F] = vh
    vha = vha.astype(bf16)
    consts = np.concatenate([
        wrep.reshape(P, -1).view(np.uint8),
        tcol.view(np.uint8),
        vha.view(np.uint8),
        np.zeros((P, 2), np.uint8),
    ], axis=1)
    assert consts.shape == (P, CONST_BYTES), consts.shape

    X = np.asarray(graph_node, f32).astype(np.float16)
    in_maps = []
    for core in range(NCORES):
        xpad = np.zeros((PADROWS, D), np.float16)
        xpad[:SHARD] = X[core * SHARD:(core + 1) * SHARD]
        # xT[dc, dp, b*512 + rc*128 + rp] = x[b*512 + 4*rp + rc, dc*128 + dp]
        # full pairs: 8-row striping (column pr*1024 + b*512 + rc*128 + rp
        # holds row pr*1024 + 8*rp + 4*b + rc)
        npr = (NBLK - 1) // 2
        v1 = xpad[:npr * 2 * BLK].reshape(npr, P, 2, 4, 4, P)
        t1 = v1.transpose(4, 5, 0, 2, 3, 1).reshape(4, P, npr * 2 * BLK)
        # trailing block: 4-row striping (column rc*128 + rp -> row 4*rp+rc)
        v2 = xpad[npr * 2 * BLK:].reshape(P, 4, 4, P)
        t2 = v2.transpose(2, 3, 1, 0).reshape(4, P, BLK)
        xT = np.ascontiguousarray(np.concatenate([t1, t2], axis=2))
        in_maps.append({"xT": xT, "consts": consts})
    return in_maps


def _gather(results):
    return np.concatenate(
        [results[c]["out"][:SHARD].astype(np.float32) for c in range(NCORES)],
        axis=0)


def _run(inputs, trace=False, **trace_kwargs):
    from concourse.bass_utils import run_bass_kernel_spmd

    nc = _get_nc()
    in_maps = _prep_inputs(**inputs)
    res = run_bass_kernel_spmd(nc, in_maps, list(range(NCORES)),
                               trace=trace, **trace_kwargs)
    return _gather(res.results), res


def kernel(**inputs) -> np.ndarray:
    out, _ = _run(inputs)
    return out


# revision 51
# speedup vs baseline: 1.0525x; 1.0525x over previous
"""Trainium2 Bass kernel for virtual-node GAT attention (gnn_message_passing).

Reference semantics (N=100000, C=64, D=512, F=256):
    gh  = graph_node @ W            # (N, F)
    vh  = virtual_node @ W          # (C, F)
    e   = gh @ a1 + (vh @ a2)^T     # (N, C)
    e   = leaky_relu(e, 0.2)
    att = softmax(e, axis=1)
    out = att @ vh                  # (N, F)

Algebraic identity: gh only enters via gh @ a1 = graph_node @ (W @ a1), so
the (N,D)@(D,F) matmul never happens. Host precomputes the tiny shared
tables w1 = W@a1 (D,), vh (C,F), t = vh@a2 (C,). The kernel is HBM-bound:
streaming x in and h' out once. Both streams ride bf16 (host casts), which
halves HBM traffic vs fp32; rel-err budget (2e-2) dwarfs bf16 noise.

Device pipeline, per 512-row block (x shipped TRANSPOSED by the host as
[4 d-chunks, 128, rows] bf16):
  PE   e^T[j, r] = sum_d w1[d] x[r, d]: 4 accumulating matmuls with
       lhsT = (w1 chunk) replicated across 64 columns, rhs = xT chunk.
       Output lands already transposed for the att matmul (no PE transpose,
       no DVE dot product). Two blocks pack one PSUM bank (partitions
       0-63 / 64-127).
  ACT  e = prelu(e^T + t) (bias=t fused, alpha honored), then exp -> bf16.
  PE   h'[r, :] = att^T.T @ [vh | 1]: the ones column makes col 256 the
       softmax denominator z (no reduction pass).
  DVE  r = 1/z; normalization fused into the PSUM->SBUF copies (split
       between ACT and DVE), writing bf16.

Host column permutation: xT column rc*128+rp holds row 4*rp+rc of its
block, so each h'-matmul output partition owns 4 consecutive HBM rows ->
2KB contiguous store packets and natural row order (no un-permute).

Sharding: rows split evenly across 8 cores (data parallel); small tables
replicated; no cross-device communication.
"""

import numpy as np

N, D, F, C = 100000, 512, 256, 64
NCORES = 8
SHARD = N // NCORES            # 12500 rows per core
P = 128
BLK = 512                      # rows per block (4 psum chunks of 128)
NBLK = 25                      # ceil(12500 / 512)
PADROWS = NBLK * BLK           # 12800
# Pairs of blocks share one PSUM logits bank: (0), (1,2), ..., (23,24).
# The leading singleton lets compute start after a 1-block first DMA group.
GROUPS_IN = [1, 2, 4, 6, 6, 6]   # blocks per input DMA instruction
GROUPS_OUT = [1, 2, 4, 6, 6, 4, 2]  # blocks per output DMA (early + small tail)
assert sum(GROUPS_IN) == NBLK and sum(GROUPS_OUT) == NBLK
CONST_BYTES = 2 * 4 * C + 4 + 2 * (F + 1) + 2   # 1032 (pad: pitch % 4 == 0)
ALPHA = 0.2

_CACHE = {}


def _build_nc():
    import concourse.bacc as bacc
    import concourse.mybir as mybir
    import concourse.tile as tile

    fp32 = mybir.dt.float32
    bf16 = mybir.dt.bfloat16
    fp16 = mybir.dt.float16
    Act = mybir.ActivationFunctionType
    Alu = mybir.AluOpType

    nc = bacc.Bacc("TRN2", target_bir_lowering=False, debug=False,
                   num_devices=NCORES)
    # x and w1 ride fp16 (same bytes as bf16, 8x finer mantissa -> the
    # logits see ~8x less quantization noise). pexp/vha stay bf16: exp can
    # reach ~5e8 which overflows fp16.
    xT = nc.dram_tensor("xT", [4, P, PADROWS], fp16, kind="ExternalInput").ap()
    # wrep fp16 [P,4,C] | tcol fp32 [P,1] | vha bf16 [P,F+1], packed as bytes
    consts = nc.dram_tensor("consts", [P, CONST_BYTES], mybir.dt.uint8,
                            kind="ExternalInput").ap()
    out = nc.dram_tensor("out", [PADROWS, F], bf16, kind="ExternalOutput").ap()

    # block -> (input group idx, local block idx); same for output groups
    gin_of, gout_of = {}, {}
    b = 0
    for g, gs in enumerate(GROUPS_IN):
        for i in range(gs):
            gin_of[b] = (g, i)
            b += 1
    b = 0
    for g, gs in enumerate(GROUPS_OUT):
        for i in range(gs):
            gout_of[b] = (g, i)
            b += 1
    gin_row0 = np.cumsum([0] + GROUPS_IN)
    gout_row0 = np.cumsum([0] + GROUPS_OUT)

    with tile.TileContext(nc) as tc:
        with (
            tc.tile_pool(name="const", bufs=1) as constp,
            tc.tile_pool(name="xin", bufs=3) as xp,
            tc.tile_pool(name="esb", bufs=3) as ep,
            tc.tile_pool(name="pexp", bufs=4) as pexpp,
            tc.tile_pool(name="rvec", bufs=6) as rp_,
            tc.tile_pool(name="osb", bufs=3) as op_,
            tc.tile_pool(name="psE", bufs=2, space="PSUM") as psE,
            tc.tile_pool(name="psH", bufs=3, space="PSUM") as psH,
        ):
            # all consts ride ONE dma on the ACT HWDGE ring (the x stream
            # owns the SP ring); typed views are bitcasts of the byte tile
            cst = constp.tile([P, CONST_BYTES], mybir.dt.uint8, name="cst")
            nc.scalar.dma_start(out=cst, in_=consts)
            wrep_sb = cst[:, 0:512].bitcast(fp16).rearrange(
                "p (c j) -> p c j", c=4)
            tcol_sb = cst[:, 512:516].bitcast(fp32)
            vha_sb = cst[:, 516:516 + 2 * (F + 1)].bitcast(bf16)

            # zeroed tile for PE-warming filler matmuls (see front())
            warm16 = constp.tile([P, BLK], fp16, name="warm16")
            nc.vector.memset(warm16, 0.0)



            xt_tiles = [None] * len(GROUPS_IN)
            osb_tiles = [None] * len(GROUPS_OUT)

            def ensure_xt(b):
                g, _ = gin_of[b]
                if xt_tiles[g] is None:
                    gs = GROUPS_IN[g]
                    t = xp.tile([P, 4, gs * BLK], fp16, tag="xt", name="xt")
                    src = xT[:, :, gin_row0[g] * BLK:(gin_row0[g] + gs) * BLK]
                    nc.sync.dma_start(out=t, in_=src.rearrange("c p r -> p c r"))
                    xt_tiles[g] = t
                return xt_tiles[g], gin_of[b][1]

            def ensure_osb(b):
                g, _ = gout_of[b]
                if osb_tiles[g] is None:
                    gs = GROUPS_OUT[g]
                    osb_tiles[g] = op_.tile([P, gs, 4, F], bf16, tag="osb",
                                            name="osb")
                return osb_tiles[g], gout_of[b][1]

            pairs = [(0,)] + [(b, b + 1) for b in range(1, NBLK, 2)]
            pex_of = {}

            def front(pi):
                # e^T matmuls + prelu + exp for pair pi. Emitted one pair
                # AHEAD of the back half so the next prelu/exp sit in the
                # ACT FIFO before this pair's copies — keeps the PE fed and
                # the HAM clock-gate warm.
                pair = pairs[pi]
                nh = len(pair)               # blocks in this psum pair
                npart = nh * C               # active psum partitions
                pse = psE.tile([P, BLK], fp32, name="pse", tag="pse")
                # PE-warming fillers: absorb the PE's wait-for-DMA/ACT gap
                # so the HAM clock gate never sees an idle window and the
                # array stays at 2.4 GHz. They write garbage into pse that
                # the first real matmul's start=True immediately resets.
                for _ in range(3):
                    nc.tensor.matmul(pse[:C, :F], warm16[:, :C],
                                     warm16[:, :F], start=True, stop=True)
                for h in range(nh):
                    xt, lb = ensure_xt(pair[h])
                    for dc in range(4):
                        nc.tensor.matmul(
                            pse[h * C:(h + 1) * C, :],
                            wrep_sb[:, dc, :],
                            xt[:, dc, lb * BLK:(lb + 1) * BLK],
                            start=(dc == 0), stop=(dc == 3))
                esb = ep.tile([P, BLK], fp32, tag="esb", name="esb")
                nc.scalar.activation(
                    out=esb[:npart, :], in_=pse[:npart, :], func=Act.Prelu,
                    bias=tcol_sb[:npart, :], scale=1.0, alpha=ALPHA)
                pex = pexpp.tile([P, BLK], bf16, tag="pex", name="pex")
                nc.scalar.activation(out=pex[:npart, :], in_=esb[:npart, :],
                                     func=Act.Exp)
                pex_of[pi] = pex

            def back(pi):
                pair = pairs[pi]
                nh = len(pair)
                pex = pex_of.pop(pi)
                for h in range(nh):
                    b = pair[h]
                    osb, ob = ensure_osb(b)
                    for cc in range(2):
                        ph = psH.tile([P, 2, BLK], fp32, name="ph", tag="ph")
                        for i in range(2):
                            rc = cc * 2 + i
                            nc.tensor.matmul(
                                ph[:, i, :F + 1],
                                pex[h * C:(h + 1) * C, rc * P:(rc + 1) * P],
                                vha_sb[h * C:(h + 1) * C, :],
                                start=True, stop=True)
                        r2 = rp_.tile([P, 2, 1], fp32)
                        nc.vector.reciprocal(r2[:, :, 0], ph[:, :, F])
                        # normalize during PSUM->SBUF copy. One tile per
                        # pair drains via ACT (2 per-chunk scaled copies);
                        # the rest drain via a single DVE pass each, with
                        # 1/z broadcast along the free dim
                        k = 2 * h + cc
                        if k == 0:
                            nc.scalar.mul(osb[:, ob, 0, :],
                                          ph[:, 0, :F], r2[:, 0, :])
                            nc.scalar.mul(osb[:, ob, 1, :],
                                          ph[:, 1, :F], r2[:, 1, :])
                        else:
                            nc.vector.scalar_tensor_tensor(
                                out=osb[:, ob, cc * 2:cc * 2 + 2, :],
                                in0=ph[:, :, :F], scalar=1.0,
                                in1=r2.broadcast_to([P, 2, F]),
                                op0=Alu.mult, op1=Alu.mult)
                    g, ob2 = gout_of[b]
                    if ob2 == GROUPS_OUT[g] - 1:
                        gs = GROUPS_OUT[g]
                        dst = out[gout_row0[g] * BLK:(gout_row0[g] + gs) * BLK, :]
                        nc.scalar.dma_start(
                            out=dst.rearrange("(b p four) f -> p b four f",
                                              four=4, p=P),
                            in_=osb_tiles[g])

            # depth-1 software pipeline: exp(k+1) sits ahead of pair k's
            # copies in the ACT FIFO so the PE's h' matmuls don't wait
            npairs = len(pairs)
            front(0)
            for pi in range(1, npairs):
                front(pi)
                back(pi - 1)
            back(npairs - 1)

    nc.compile()
    return nc


def _get_nc():
    if "nc" not in _CACHE:
        _CACHE["nc"] = _build_nc()
    return _CACHE["nc"]


def _prep_inputs(graph_node, virtual_node, W, a):
    import ml_dtypes
    f32 = np.float32
    bf16 = ml_dtypes.bfloat16
    W = np.asarray(W, f32)
    a = np.asarray(a, f32)
    a1 = a[:F, 0]
    a2 = a[F:, 0]
    w1 = (W @ a1).astype(f32)                             # (D,)
    vh = (np.asarray(virtual_node, f32) @ W).astype(f32)  # (C, F)
    t = (vh @ a2).astype(f32)                             # (C,)

    # wrep[p, dc, j] = w1[dc*128 + p] for all j (broadcast across columns)
    wrep = np.ascontiguousarray(
        np.broadcast_to(w1.reshape(4, P).T[:, :, None], (P, 4, C))
    ).astype(np.float16)
    tcol = np.ascontiguousarray(np.concatenate([t, t])[:, None], dtype=f32)
    vha = np.ones((P, F + 1), f32)
    vha[:C, :F] = vh
    vha[C:, :F] = vh
    vha = vha.astype(bf16)
    consts = np.concatenate([
        wrep.reshape(P, -1).view(np.uint8),
        tcol.view(np.uint8),
        vha.view(np.uint8),
        np.zeros((P, 2), np.uint8),
    ], axis=1)
    assert consts.shape == (P, CONST_BYTES), consts.shape

    X = np.asarray(graph_node, f32).astype(np.float16)
    in_maps = []
    for core in range(NCORES):
        xpad = np.zeros((PADROWS, D), np.float16)
        xpad[:SHARD] = X[core * SHARD:(core + 1) * SHARD]
        # xT[dc, dp, b*512 + rc*128 + rp] = x[b*512 + 4*rp + rc, dc*128 + dp]
        v = xpad.reshape(NBLK, P, 4, 4, P)       # [b, rp, rc, dc, dp]
        xT = np.ascontiguousarray(v.transpose(3, 4, 0, 2, 1)).reshape(
            4, P, PADROWS)
        in_maps.append({"xT": xT, "consts": consts})
    return in_maps


def _gather(results):
    return np.concatenate(
        [results[c]["out"][:SHARD].astype(np.float32) for c in range(NCORES)],
        axis=0)


def _run(inputs, trace=False, **trace_kwargs):
    from concourse.bass_utils import run_bass_kernel_spmd

    nc = _get_nc()
    in_maps = _prep_inputs(**inputs)
    res = run_bass_kernel_spmd(nc, in_maps, list(range(NCORES)),
                               trace=trace, **trace_kwargs)
    return _gather(res.results), res


def kernel(**inputs) -> np.ndarray:
    out, _ = _run(inputs)
    return out
